# revision 1
# baseline (speedup 1.0000x reference)
"""CapsuleConv2d (3-iteration dynamic routing) Bass kernel for 8 TRN2 cores.

Strategy (data-parallel over batch, 2 images per core):
  - priors[l, ij, o, u, f] computed by PE per 128-location tile:
    stationary = padded-x window [32=(f,d), 128 locs], moving = structured
    weight constants [32, (o,u,f)] per kernel tap ij.  s0 = 0.25*sum_r priors
    accumulated by PE in the same pass.
  - routing in "natural" layout (locations on partitions): DVE does the
    broadcast-multiplies + segmented reduces, ACT does exp/square/sqrt and
    PSUM->SBUF copies, PE transposes the final [128 locs, 32 ch] result for
    channel-major DMA out.
"""
import numpy as np

import concourse.bass as bass
import concourse.bacc as bacc
import concourse.tile as tile
import concourse.mybir as mybir
import concourse.bass_utils as bass_utils

# All ACT functions we use (Exp, Ln, Square, Copy, ...) live together in the
# "natural_log_exp_and_others" table set, but bacc's table-load pass picks a
# per-function set greedily (Ln -> natural_log, Exp -> exp_and_others),
# thrashing ~2.7us table loads between them.  Restrict Exp/Ln to the combined
# set so a single load covers the whole kernel.
_orig_get_tables = bacc.get_activation_tables
_AFT = mybir.ActivationFunctionType


def _patched_get_tables(arch):
    tables = dict(_orig_get_tables(arch))
    for name, funcs in tables.items():
        if name != "natural_log_exp_and_others":
            tables[name] = funcs - {_AFT.Exp, _AFT.Ln}
    return tables


bacc.get_activation_tables = _patched_get_tables

# ---- problem constants (hardcoded; must match setup_inputs) ----
O, F, U, D = 4, 4, 8, 8
KH = KW = 3
NIJ = KH * KW
H = W = 64
C = 32
N_FULL = 16
N_CORES = 8
IMG_PER_CORE = N_FULL // N_CORES
HP, WP = H + 2, W + 2              # padded input
LT_ROWS = 2                        # output rows per 128-loc tile
NLT = H // LT_ROWS                 # 32 loc-tiles per image
ST_LT = 4                          # loc-tiles per super-tile (512 locs)
NST = NLT // ST_LT                 # 8 super-tiles per image
PB = 2                             # super-tiles batched per routing pass
PLT = PB * ST_LT                   # loc-tiles per routing pass (8)
EPS = 1e-12

f32 = mybir.dt.float32
AL = mybir.AluOpType
AF = mybir.ActivationFunctionType
AX = mybir.AxisListType

_COMPILED = None


def _build(dump=False, repeat=1):
    nc = bacc.Bacc("TRN2", target_bir_lowering=False, debug=False)

    dbg = {}
    if dump:
        for name, shape in [("dbg_P", [128, ST_LT * 1152]),
                            ("dbg_s0", [128, ST_LT * 32]),
                            ("dbg_v0", [128, ST_LT * 32]),
                            ("dbg_b1", [128, ST_LT * 144]),
                            ("dbg_E1", [128, ST_LT * 144]),
                            ("dbg_s1", [128, ST_LT * 32])]:
            dbg[name] = nc.dram_tensor(name, shape, f32,
                                       kind="ExternalOutput").ap()

    xin_d = nc.dram_tensor("xin", [IMG_PER_CORE, C, H * W], f32,
                           kind="ExternalInput").ap()
    wmov_d = nc.dram_tensor("wmov", [C, NIJ * 128], f32,
                            kind="ExternalInput").ap()
    wsum_d = nc.dram_tensor("wsum", [C, NIJ * 32], f32,
                            kind="ExternalInput").ap()
    ident_d = nc.dram_tensor("ident", [128, 128], f32,
                             kind="ExternalInput").ap()
    out_d = nc.dram_tensor("out", [IMG_PER_CORE, C, H * W], f32,
                           kind="ExternalOutput").ap()

    with tile.TileContext(nc) as tc:
        with tc.tile_pool(name="const", bufs=1) as cpool, \
             tc.tile_pool(name="xpad", bufs=1) as xpool, \
             tc.tile_pool(name="stage", bufs=1) as spool, \
             tc.tile_pool(name="pst", bufs=4) as ppool, \
             tc.tile_pool(name="gh", bufs=2) as ghpool, \
             tc.tile_pool(name="small", bufs=2) as smpool, \
             tc.tile_pool(name="ppri", bufs=2, space="PSUM") as ppri, \
             tc.tile_pool(name="ps0", bufs=1, space="PSUM") as ps0, \
             tc.tile_pool(name="ptp", bufs=1, space="PSUM") as ptp:

            wmov_s = cpool.tile([C, NIJ * 128], f32, tag="wmov")
            wsum_s = cpool.tile([C, NIJ * 32], f32, tag="wsum")
            ident_s = cpool.tile([128, 128], f32, tag="ident")
            eps_s = cpool.tile([128, 1], f32, tag="eps")
            nc.sync.dma_start(wmov_s[:], wmov_d[:])
            nc.sync.dma_start(wsum_s[:], wsum_d[:])
            nc.sync.dma_start(ident_s[:], ident_d[:])
            nc.gpsimd.memset(eps_s[:], EPS)

            for img in range(IMG_PER_CORE):
                # one shared xpad slot: image n+1's fill overlaps image n's
                # routing tail (P-production finishes ~100us early)
                xp = xpool.tile([C, HP * WP], f32, tag="xpad")
                nc.gpsimd.memset(xp[:], 0.0)
                xv = xp[:].rearrange("p (h w) -> p h w", h=HP, w=WP)
                nc.sync.dma_start(
                    xv[:, 1:1 + H, 1:1 + W],
                    xin_d[img].rearrange("p (h w) -> p h w", h=H, w=W))
                xpads = {img: xv}
                stage = spool.tile([C, H * W], f32, tag="stage")
                for pr_rep in range((NST // PB) * repeat):
                    pr = pr_rep % (NST // PB)
                    # produce priors for PB super-tiles; small per-location
                    # tensors are batched across the pair to amortize DVE
                    # per-instruction overhead.
                    P_sts = []
                    s0_st = smpool.tile([128, PLT * 32], f32, tag="s0")
                    for half in range(PB):
                        st = pr * PB + half
                        P_st = ppool.tile([128, ST_LT * 1152], f32, tag="P")
                        P_sts.append(P_st)
                        for lt in range(ST_LT):
                            r0 = (st * ST_LT + lt) * LT_ROWS
                            glt = half * ST_LT + lt
                            pp = ppri.tile([128, 1152], f32, tag="ppri")
                            s0p = ps0.tile([128, 32], f32, tag="s0p")
                            for ij in range(NIJ):
                                i, j = ij // KW, ij % KW
                                for r in range(LT_ROWS):
                                    xw = xpads[img][:, r0 + i + r, j:j + W]
                                    prow = slice(r * W, (r + 1) * W)
                                    nc.tensor.matmul(
                                        pp[prow, ij * 128:(ij + 1) * 128],
                                        xw,
                                        wmov_s[:, ij * 128:(ij + 1) * 128],
                                        start=True, stop=True)
                                    nc.tensor.matmul(
                                        s0p[prow], xw,
                                        wsum_s[:, ij * 32:(ij + 1) * 32],
                                        start=(ij == 0),
                                        stop=(ij == NIJ - 1))
                            nc.scalar.copy(
                                P_st[:, lt * 1152:(lt + 1) * 1152], pp[:])
                            nc.scalar.copy(
                                s0_st[:, glt * 32:(glt + 1) * 32], s0p[:])

                    # ------- routing on this super-tile pair -------
                    def P5(half, lt):
                        return P_sts[half][:, lt * 1152:(lt + 1) *
                                           1152].rearrange(
                            "p (ij o u f) -> p ij o u f", ij=NIJ, o=O, u=U,
                            f=F)

                    def squash(s_st, tagp, newton=True):
                        # s_st: [128, (lt, o, u)]; returns v [128, (lt,o,u)]
                        # sqrt & reciprocals go through the ACT exp/ln
                        # tables (single table set, no DVE reciprocal);
                        # one Newton step restores sqrt to fp32 accuracy.
                        # newton=False skips it where the ~5e-6 table error
                        # is not amplified (the final squash: error passes
                        # straight to the output instead of through the
                        # routing logits).
                        sq = smpool.tile([128, PLT * 32], f32,
                                         tag=f"sq{tagp}")
                        nc.scalar.activation(sq[:], s_st[:], AF.Square)
                        n2 = smpool.tile([128, PLT * O], f32,
                                         tag=f"n2{tagp}")
                        nc.vector.tensor_reduce(
                            n2[:],
                            sq[:].rearrange("p (lt o u) -> p lt o u",
                                            lt=PLT, o=O, u=U),
                            AX.X, AL.add)
                        Ltile = smpool.tile([128, PLT * O], f32,
                                            tag=f"L{tagp}")
                        nc.scalar.activation(Ltile[:], n2[:], AF.Ln,
                                             bias=eps_s[:])
                        t_ = smpool.tile([128, PLT * O], f32,
                                         tag=f"t{tagp}")
                        nc.scalar.activation(t_[:], Ltile[:], AF.Exp,
                                             scale=0.5)
                        if newton:
                            r5 = smpool.tile([128, PLT * O], f32,
                                             tag=f"r5{tagp}")
                            # true Newton needs 1/t of the current t —
                            # exact DVE reciprocal (exp(-ln t) tables are
                            # ~1e-5 off)
                            nc.vector.reciprocal(r5[:], t_[:])
                            xr = smpool.tile([128, PLT * O], f32,
                                             tag=f"xr{tagp}")
                            # xr = n2 * (0.5/t)  (eps negligible: fi -> 0
                            # as n2 -> 0 regardless)
                            nc.vector.scalar_tensor_tensor(
                                xr[:], r5[:], 0.5, n2[:], AL.mult, AL.mult)
                            # t = 0.5*t + xr   (Newton)
                            nc.vector.scalar_tensor_tensor(
                                t_[:], t_[:], 0.5, xr[:], AL.mult, AL.add)
                        # w = (1+n2)*t;  fi = n2 / w
                        pw = smpool.tile([128, PLT * O], f32,
                                         tag=f"pw{tagp}")
                        nc.vector.scalar_tensor_tensor(
                            pw[:], n2[:], 1.0, t_[:], AL.add, AL.mult)
                        rw = smpool.tile([128, PLT * O], f32,
                                         tag=f"rw{tagp}")
                        nc.vector.reciprocal(rw[:], pw[:])
                        fi = smpool.tile([128, PLT * O], f32,
                                         tag=f"fi{tagp}")
                        nc.vector.tensor_tensor(fi[:], n2[:], rw[:], AL.mult)
                        v = smpool.tile([128, PLT * 32], f32,
                                        tag=f"v{tagp}")
                        fib = fi[:].rearrange("p (lt o) -> p lt o",
                                              lt=PLT).unsqueeze(3)
                        nc.vector.tensor_tensor(
                            v[:].rearrange("p (lt o u) -> p lt o u",
                                           lt=PLT, o=O, u=U),
                            s_st[:].rearrange("p (lt o u) -> p lt o u",
                                              lt=PLT, o=O, u=U),
                            fib.broadcast_to((128, PLT, O, U)), AL.mult)
                        return v

                    is_dbg = dump and img == 0 and pr == 0
                    if is_dbg:
                        nc.sync.dma_start(dbg["dbg_P"][:], P_sts[0][:])
                        nc.sync.dma_start(dbg["dbg_s0"][:],
                                          s0_st[:, :ST_LT * 32])

                    v = squash(s0_st, "0")
                    if is_dbg:
                        nc.sync.dma_start(dbg["dbg_v0"][:],
                                          v[:, :ST_LT * 32])

                    # b1[l, (half, lt, ij, o, f)] = sum_u P * v0
                    b_st = smpool.tile([128, PLT * 144], f32, tag="b")
                    hred = smpool.tile([128, PLT * 144], f32, tag="hred")
                    for it in range(3):
                        if it > 0:
                            # E = exp(b); Z = sum_o E; E' = E / Z
                            E = smpool.tile([128, PLT * 144], f32, tag="E")
                            nc.scalar.activation(E[:], b_st[:], AF.Exp)
                            Ev = E[:].rearrange(
                                "p (lt ij o f) -> p lt ij o f", lt=PLT,
                                ij=NIJ, o=O, f=F)
                            Z = smpool.tile([128, PLT * 36], f32, tag="Z")
                            nc.vector.tensor_reduce(
                                Z[:], Ev.transpose([0, 1, 2, 4, 3]), AX.X,
                                AL.add)
                            Zi = smpool.tile([128, PLT * 36], f32,
                                             tag="Zi")
                            nc.vector.reciprocal(Zi[:], Z[:])
                            Zib = Zi[:].rearrange(
                                "p (lt ij f) -> p lt ij f", lt=PLT,
                                ij=NIJ).unsqueeze(3).broadcast_to(
                                    (128, PLT, NIJ, O, F))
                            nc.vector.tensor_tensor(Ev, Ev, Zib, AL.mult)
                            s_st = smpool.tile([128, PLT * 32], f32,
                                               tag="s")
                            KK = ST_LT * NIJ
                            for half in range(PB):
                                # G = E' * P, one op per half ((lt, ij)
                                # collapses to one affine axis k)
                                G = ghpool.tile([128, ST_LT * 1152], f32,
                                                tag="gh")
                                Gk = G[:].rearrange(
                                    "p (k o u f) -> p k o u f", k=KK, o=O,
                                    u=U, f=F)
                                Pk = P_sts[half][:].rearrange(
                                    "p (k o u f) -> p k o u f", k=KK, o=O,
                                    u=U, f=F)
                                Ek = E[:, half * ST_LT * 144:(half + 1) *
                                       ST_LT * 144].rearrange(
                                    "p (k o f) -> p k o f", k=KK,
                                    o=O).unsqueeze(3).broadcast_to(
                                        (128, KK, O, U, F))
                                nc.vector.tensor_tensor(Gk, Pk, Ek, AL.mult)
                                # s[l, (half, lt, o, u)] = sum_{ij,f} G
                                for lt in range(ST_LT):
                                    glt = half * ST_LT + lt
                                    G5 = G[:, lt * 1152:(lt + 1) *
                                           1152].rearrange(
                                               "p (ij o u f) -> p ij o u f",
                                               ij=NIJ, o=O, u=U, f=F)
                                    nc.vector.tensor_reduce(
                                        s_st[:, glt * 32:(glt + 1) * 32],
                                        G5.transpose([0, 2, 3, 1, 4]),
                                        AX.XY, AL.add)
                            if is_dbg and it == 1:
                                nc.sync.dma_start(dbg["dbg_E1"][:],
                                                  E[:, :ST_LT * 144])
                                nc.sync.dma_start(dbg["dbg_s1"][:],
                                                  s_st[:, :ST_LT * 32])
                            v = squash(s_st, "12")
                        if it < 2:
                            # accumulate logits: b += sum_u P * v
                            dst = b_st if it == 0 else hred
                            for half in range(PB):
                                Hst = ghpool.tile([128, ST_LT * 1152], f32,
                                                  tag="gh")
                                # H = P * v_bcast per lt (v's broadcast AP
                                # needs [ij, (o,u), f] = 3 AP dims; adding
                                # lt would exceed the DVE TENSOR3D limit).
                                for lt in range(ST_LT):
                                    glt = half * ST_LT + lt
                                    H5 = Hst[:, lt * 1152:(lt + 1) *
                                             1152].rearrange(
                                                 "p (ij o u f) -> "
                                                 "p ij o u f",
                                                 ij=NIJ, o=O, u=U, f=F)
                                    vb = v[:, glt * 32:(glt + 1) *
                                           32].rearrange(
                                        "p (o u) -> p o u",
                                        o=O).unsqueeze(1).unsqueeze(
                                            4).broadcast_to(
                                                (128, NIJ, O, U, F))
                                    nc.vector.tensor_tensor(
                                        H5, P5(half, lt), vb, AL.mult)
                                # one segmented reduce over u per half
                                Hk = Hst[:].rearrange(
                                    "p (k o u f) -> p k o u f",
                                    k=ST_LT * NIJ, o=O, u=U, f=F)
                                nc.vector.tensor_reduce(
                                    dst[:, half * ST_LT * 144:(half + 1) *
                                        ST_LT * 144],
                                    Hk.transpose([0, 1, 2, 4, 3]),
                                    AX.X, AL.add)
                            if it == 0 and is_dbg:
                                nc.sync.dma_start(dbg["dbg_b1"][:],
                                                  b_st[:, :ST_LT * 144])
                            if it == 1:
                                nc.vector.tensor_tensor(b_st[:], b_st[:],
                                                        hred[:], AL.add)

                    # v now holds squash(s2): transpose to [32, locs] & stage
                    for glt in range(PLT):
                        r0 = (pr * PLT + glt) * LT_ROWS
                        tp = ptp.tile([32, 128], f32, tag="tp")
                        nc.tensor.transpose(tp[:],
                                            v[:, glt * 32:(glt + 1) * 32],
                                            ident_s[:])
                        nc.scalar.copy(
                            stage[:, r0 * W:r0 * W + LT_ROWS * W], tp[:])

                nc.sync.dma_start(out_d[img], stage[:])

    nc.compile()
    return nc


def _get_compiled():
    global _COMPILED
    if _COMPILED is None:
        _COMPILED = _build()
    return _COMPILED


def _make_consts(weight):
    w = np.asarray(weight, dtype=np.float32)  # [o, f, i, j, u, d]
    wmov = np.zeros((C, NIJ * 128), dtype=np.float32)
    wsum = np.zeros((C, NIJ * 32), dtype=np.float32)
    for o in range(O):
        for f in range(F):
            for ij in range(NIJ):
                i, j = ij // KW, ij % KW
                for u in range(U):
                    for d in range(D):
                        wmov[f * D + d,
                             ij * 128 + o * 32 + u * 4 + f] = w[o, f, i, j,
                                                                u, d]
                        wsum[f * D + d,
                             ij * 32 + o * 8 + u] = 0.25 * w[o, f, i, j, u,
                                                             d]
    return wmov, wsum


def kernel(x, weight):
    x = np.ascontiguousarray(np.asarray(x, dtype=np.float32))
    wmov, wsum = _make_consts(weight)
    ident = np.eye(128, dtype=np.float32)

    nc = _get_compiled()
    in_maps = []
    for c in range(N_CORES):
        xin = x[c * IMG_PER_CORE:(c + 1) * IMG_PER_CORE].reshape(
            IMG_PER_CORE, C, H * W)
        in_maps.append({
            "xin": np.ascontiguousarray(xin),
            "wmov": wmov,
            "wsum": wsum,
            "ident": ident,
        })
    res = bass_utils.run_bass_kernel_spmd(nc, in_maps,
                                          core_ids=list(range(N_CORES)))
    out = np.empty((N_FULL, C, H, W), dtype=np.float32)
    for c in range(N_CORES):
        out[c * IMG_PER_CORE:(c + 1) * IMG_PER_CORE] = res.results[c][
            "out"].reshape(IMG_PER_CORE, C, H, W)
    return out



# revision 28
# speedup vs baseline: 1.3910x; 1.3910x over previous
"""CapsuleConv2d (3-iteration dynamic routing) Bass kernel for 8 TRN2 cores.

Strategy (data-parallel over batch, 2 images per core):
  - priors P[l, (lt,o,ij,f,u)] by PE per 1024-location pass: stationary =
    padded-x window [64=(hi,lo dup of f,d), 128 locs], moving = structured
    weight constants (hi+lo fp16 split stacked on K) per kernel tap.
    s0 = 0.25*sum_i P accumulated by PE in the same pass via wsum moving.
  - x cast to fp16; w split w_hi+w_lo (both fp16) so the only priors error
    is the x cast (~9e-3 rel at the output, tolerance 2e-2).
  - routing in natural layout (locations on partitions): the four P-sized
    multiplies run on DVE in fp16 at the 2x_1p rate (all operands 2-byte,
    innermost step-1); the four P-sized segmented reductions are fp32
    tensor_reduce ops split between DVE and Pool (gpsimd); ACT does exp,
    PSUM->SBUF copies and the probs-over-u broadcast (probsU) that keeps
    the G-multiply 2x-eligible.
  - logits b stay fp32 end to end (fp16 logits flip routing decisions).
"""
import numpy as np

import concourse.bass as bass
import concourse.bacc as bacc
import concourse.tile as tile
import concourse.mybir as mybir
import concourse.bass_utils as bass_utils

# All ACT functions we use (Exp, Ln, Square, Copy, ...) live together in the
# "natural_log_exp_and_others" table set, but bacc's table-load pass picks a
# per-function set greedily (Ln -> natural_log, Exp -> exp_and_others),
# thrashing ~2.7us table loads between them.  Restrict Exp/Ln to the combined
# set so a single load covers the whole kernel.
_orig_get_tables = bacc.get_activation_tables
_AFT = mybir.ActivationFunctionType


def _patched_get_tables(arch):
    tables = dict(_orig_get_tables(arch))
    for name, funcs in tables.items():
        if name != "natural_log_exp_and_others":
            tables[name] = funcs - {_AFT.Exp, _AFT.Ln}
    return tables


bacc.get_activation_tables = _patched_get_tables

# ---- problem constants (hardcoded; must match setup_inputs) ----
O, F, U, D = 4, 4, 8, 8
KH = KW = 3
NIJ = KH * KW
IJF = NIJ * F                      # 36 in-capsules
PF = O * IJF * U                   # 1152 prior elems per location
H = W = 64
C = 32
N_FULL = 16
N_CORES = 8
IMG_PER_CORE = N_FULL // N_CORES
HP, WP = H + 2, W + 2              # padded input
LT_ROWS = 2                        # output rows per 128-loc tile
NLT = H // LT_ROWS                 # 32 loc-tiles per image
PLT = 4                            # loc-tiles per routing pass (512 locs)
NPASS = NLT // PLT                 # 8 passes per image
EPS = 1e-12

f32 = mybir.dt.float32
f16 = mybir.dt.float16
AL = mybir.AluOpType
AF = mybir.ActivationFunctionType
AX = mybir.AxisListType

_COMPILED = None


def _build(dump=False, repeat=1):
    nc = bacc.Bacc("TRN2", target_bir_lowering=False, debug=False)

    dbg = {}
    if dump:
        for name, shape in [("dbg_P", [128, PLT * PF]),
                            ("dbg_s0", [128, PLT * 32]),
                            ("dbg_v0", [128, PLT * 32]),
                            ("dbg_b1", [128, PLT * 144]),
                            ("dbg_pr1", [128, PLT * 144]),
                            ("dbg_s1", [128, PLT * 32])]:
            dbg[name] = nc.dram_tensor(name, shape, f32,
                                       kind="ExternalOutput").ap()

    xin_d = nc.dram_tensor("xin", [IMG_PER_CORE, C, H * W], f32,
                           kind="ExternalInput").ap()
    # hi/lo fp16 split of the weights, stacked on K (the priors matmul
    # contracts over 64 = (f,d) x {hi,lo}); column order per tap: o*32+f*8+u
    wmov_d = nc.dram_tensor("wmov", [2 * C, NIJ * 128], f16,
                            kind="ExternalInput").ap()
    wsum_d = nc.dram_tensor("wsum", [2 * C, NIJ * 32], f16,
                            kind="ExternalInput").ap()
    ident_d = nc.dram_tensor("ident", [128, 128], f32,
                             kind="ExternalInput").ap()
    out_d = nc.dram_tensor("out", [IMG_PER_CORE, C, H * W], f32,
                           kind="ExternalOutput").ap()

    with tile.TileContext(nc) as tc:
        with tc.tile_pool(name="const", bufs=1) as cpool, \
             tc.tile_pool(name="xpad", bufs=1) as xpool, \
             tc.tile_pool(name="stage", bufs=1) as spool, \
             tc.tile_pool(name="pst", bufs=2) as ppool, \
             tc.tile_pool(name="wprod", bufs=1) as wpool, \
             tc.tile_pool(name="wbig", bufs=4) as wbpool, \
             tc.tile_pool(name="small", bufs=2) as smpool, \
             tc.tile_pool(name="ppri", bufs=2, space="PSUM") as ppri, \
             tc.tile_pool(name="ps0", bufs=1, space="PSUM") as ps0, \
             tc.tile_pool(name="ptp", bufs=1, space="PSUM") as ptp:

            wmov_s = cpool.tile([2 * C, NIJ * 128], f16, tag="wmov")
            wsum_s = cpool.tile([2 * C, NIJ * 32], f16, tag="wsum")
            ident_s = cpool.tile([128, 128], f32, tag="ident")
            eps_s = cpool.tile([128, 1], f32, tag="eps")
            nc.sync.dma_start(wmov_s[:], wmov_d[:])
            nc.sync.dma_start(wsum_s[:], wsum_d[:])
            nc.sync.dma_start(ident_s[:], ident_d[:])
            nc.gpsimd.memset(eps_s[:], EPS)

            for img in range(IMG_PER_CORE):
                # fp32 landing pad for the DMA, then fp16 cast duplicated on
                # partitions 0-31 / 32-63 (hi/lo K-stacked priors matmuls).
                xp = xpool.tile([C, HP * WP], f32, tag="xpad")
                nc.gpsimd.memset(xp[:], 0.0)
                xv = xp[:].rearrange("p (h w) -> p h w", h=HP, w=WP)
                nc.sync.dma_start(
                    xv[:, 1:1 + H, 1:1 + W],
                    xin_d[img].rearrange("p (h w) -> p h w", h=H, w=W))
                x16 = xpool.tile([2 * C, HP * WP], f16, tag="x16")
                nc.scalar.copy(x16[0:C, :], xp[:])
                nc.scalar.copy(x16[C:2 * C, :], xp[:])
                x16v = x16[:].rearrange("p (h w) -> p h w", h=HP, w=WP)

                stage = spool.tile([C, H * W], f32, tag="stage")

                def squash(s_t, tagp, out_dtype):
                    # s_t: [128, (lt, o, u)] fp32; returns v (out_dtype)
                    sq = smpool.tile([128, PLT * 32], f32, tag=f"sq{tagp}")
                    nc.scalar.activation(sq[:], s_t[:], AF.Square)
                    n2 = smpool.tile([128, PLT * O], f32, tag=f"n2{tagp}")
                    nc.vector.tensor_reduce(
                        n2[:],
                        sq[:].rearrange("p (lt o u) -> p lt o u",
                                        lt=PLT, o=O, u=U),
                        AX.X, AL.add)
                    Ltile = smpool.tile([128, PLT * O], f32, tag=f"L{tagp}")
                    nc.scalar.activation(Ltile[:], n2[:], AF.Ln,
                                         bias=eps_s[:])
                    t_ = smpool.tile([128, PLT * O], f32, tag=f"t{tagp}")
                    nc.scalar.activation(t_[:], Ltile[:], AF.Exp, scale=0.5)
                    pw = smpool.tile([128, PLT * O], f32, tag=f"pw{tagp}")
                    nc.vector.scalar_tensor_tensor(
                        pw[:], n2[:], 1.0, t_[:], AL.add, AL.mult)
                    rw = smpool.tile([128, PLT * O], f32, tag=f"rw{tagp}")
                    nc.vector.reciprocal(rw[:], pw[:])
                    fi = smpool.tile([128, PLT * O], f32, tag=f"fi{tagp}")
                    nc.vector.tensor_tensor(fi[:], n2[:], rw[:], AL.mult)
                    v = smpool.tile([128, PLT * 32], out_dtype,
                                    tag=f"v{tagp}")
                    fib = fi[:].rearrange("p (lt o) -> p lt o",
                                          lt=PLT).unsqueeze(3)
                    nc.vector.tensor_tensor(
                        v[:].rearrange("p (lt o u) -> p lt o u",
                                       lt=PLT, o=O, u=U),
                        s_t[:].rearrange("p (lt o u) -> p lt o u",
                                         lt=PLT, o=O, u=U),
                        fib.broadcast_to((128, PLT, O, U)), AL.mult)
                    return v

                def produce(pr):
                    # ---- priors: P1 [128, (lt, o, ij, f, u)] fp16 ----
                    P1 = ppool.tile([128, PLT * PF], f16, tag="P1")
                    P2 = ppool.tile([128, PLT * PF], f16, tag="P2")
                    s0ps = ps0.tile([128, PLT * 32], f32, tag="s0p")
                    for lt in range(PLT):
                        r0 = (pr * PLT + lt) * LT_ROWS
                        pp = ppri.tile([128, NIJ * 128], f32, tag="ppri")
                        for ij in range(NIJ):
                            i, j = ij // KW, ij % KW
                            for r in range(LT_ROWS):
                                # stationary: one 64-wide row, hi/lo dup
                                # on K; out rows r*64..r*64+63
                                xw = x16v[:, r0 + i + r, j:j + W]
                                nc.tensor.matmul(
                                    pp[r * W:(r + 1) * W,
                                       ij * 128:(ij + 1) * 128],
                                    xw,
                                    wmov_s[:, ij * 128:(ij + 1) * 128],
                                    start=True, stop=True)
                                nc.tensor.matmul(
                                    s0ps[r * W:(r + 1) * W,
                                         lt * 32:(lt + 1) * 32], xw,
                                    wsum_s[:, ij * 32:(ij + 1) * 32],
                                    start=(ij == 0),
                                    stop=(ij == NIJ - 1))
                        # PSUM chunk (ij, o, f, u) -> two SBUF fp16 copies:
                        # P1 (o, ij, f, u) for the H-multiply (v bcast
                        # over ijf, u innermost step-1) and P2 (o, u, ij,
                        # f) for the G-multiply (probs bcast over u).
                        ppv = pp[:].rearrange(
                            "p (ij o f u) -> p ij o f u",
                            ij=NIJ, o=O, f=F, u=U)
                        nc.scalar.copy(
                            P1[:, lt * PF:(lt + 1) * PF].rearrange(
                                "p (o ij fu) -> p ij o fu", o=O, ij=NIJ,
                                fu=F * U),
                            pp[:].rearrange("p (ij o fu) -> p ij o fu",
                                            ij=NIJ, o=O, fu=F * U))
                        # ACT APs are limited to 3 free dims: emit the
                        # (ij,o,f,u)->(o,u,ij,f) permute as one copy per o
                        for o in range(O):
                            nc.scalar.copy(
                                P2[:, lt * PF + o * IJF * U:
                                   lt * PF + (o + 1) * IJF * U].rearrange(
                                    "p (u ij f) -> p ij f u", u=U,
                                    ij=NIJ, f=F),
                                ppv[:, :, o])
                    s0 = smpool.tile([128, PLT * 32], f32, tag="s0")
                    nc.scalar.copy(s0[:], s0ps[:])
                    return P1, P2, s0

                def routing_gen(pr, P1, P2, s0, is_dbg):
                    P4 = P1[:].rearrange("p (lt o ijf u) -> p lt o ijf u",
                                         lt=PLT, o=O, ijf=IJF, u=U)
                    P2v = P2[:].rearrange(
                        "p (lt o u ijf) -> p lt o u ijf",
                        lt=PLT, o=O, u=U, ijf=IJF)
                    if is_dbg:
                        dbgP = smpool.tile([128, PLT * PF], f32, tag="dbgP")
                        nc.vector.tensor_copy(dbgP[:], P1[:])
                        nc.sync.dma_start(dbg["dbg_P"][:], dbgP[:])
                        nc.sync.dma_start(dbg["dbg_s0"][:], s0[:])

                    v = squash(s0, "0", f16)
                    yield
                    if is_dbg:
                        dbgv = smpool.tile([128, PLT * 32], f32, tag="dbgv")
                        nc.vector.tensor_copy(dbgv[:], v[:])
                        nc.sync.dma_start(dbg["dbg_v0"][:], dbgv[:])

                    # b logits [128, (lt, o, ij, f)] fp32
                    b_t = smpool.tile([128, PLT * 144], f32, tag="b")

                    for it in range(3):
                        if it > 0:
                            # softmax over o: E = exp(b) on ACT; Z = sum_o
                            # E as 2-level tree on Pool; probs = E/Z (Pool)
                            E = smpool.tile([128, PLT * 144], f32,
                                            tag=f"E{it}")
                            nc.scalar.activation(E[:], b_t[:], AF.Exp)
                            yield
                            Ev = E[:].rearrange(
                                "p (lt o ijf) -> p lt o ijf",
                                lt=PLT, o=O, ijf=IJF)
                            Zt = smpool.tile([128, PLT * 2 * IJF], f32,
                                             tag=f"Zt{it}")
                            Ztv = Zt[:].rearrange(
                                "p (lt o2 ijf) -> p lt o2 ijf",
                                lt=PLT, o2=2, ijf=IJF)
                            nc.vector.tensor_tensor(
                                Ztv, Ev[:, :, 0:2, :], Ev[:, :, 2:4, :],
                                AL.add)
                            Z = smpool.tile([128, PLT * 36], f32,
                                            tag=f"Z{it}")
                            nc.vector.tensor_tensor(
                                Z[:].rearrange("p (lt ijf) -> p lt ijf",
                                               lt=PLT).unsqueeze(2),
                                Ztv[:, :, 0:1, :], Ztv[:, :, 1:2, :],
                                AL.add)
                            Zi = smpool.tile([128, PLT * 36], f32,
                                             tag=f"Zi{it}")
                            nc.vector.reciprocal(Zi[:], Z[:])
                            yield
                            probs = smpool.tile([128, PLT * 144], f16,
                                                tag=f"probs{it}")
                            Zib = Zi[:].rearrange(
                                "p (lt ijf) -> p lt ijf",
                                lt=PLT).unsqueeze(2).broadcast_to(
                                    (128, PLT, O, IJF))
                            nc.gpsimd.tensor_tensor(
                                probs[:].rearrange(
                                    "p (lt o ijf) -> p lt o ijf",
                                    lt=PLT, o=O, ijf=IJF),
                                Ev, Zib, AL.mult)
                            yield
                            if is_dbg and it == 1:
                                dbgp = smpool.tile([128, PLT * 144], f32,
                                                   tag="dbgp")
                                nc.vector.tensor_copy(dbgp[:], probs[:])
                                nc.sync.dma_start(dbg["dbg_pr1"][:],
                                                  dbgp[:])
                            # G = P2 * probs_bcast (fp16 2x, ijf
                            # innermost); s = sum_ijf G via fp16 tree
                            WG = wbpool.tile([128, PLT * PF], f16, tag="W")
                            WGv = WG[:].rearrange(
                                "p (lt o u ijf) -> p lt o u ijf",
                                lt=PLT, o=O, u=U, ijf=IJF)
                            prb = probs[:].rearrange(
                                "p (lt o ijf) -> p lt o ijf", lt=PLT,
                                o=O).unsqueeze(3).broadcast_to(
                                    (128, PLT, O, U, IJF))
                            nc.vector.tensor_tensor(WGv, P2v, prb, AL.mult)
                            yield
                            WGk = WG[:].rearrange(
                                "p (ltou ijf) -> p ltou ijf", ijf=IJF)
                            TG = wpool.tile([128, PLT * 32 * 18], f16,
                                            tag=f"TG{it}")
                            TGv = TG[:].rearrange(
                                "p (ltou k) -> p ltou k", k=18)
                            nc.vector.tensor_tensor(
                                TGv, WGk[:, :, 0:18], WGk[:, :, 18:36],
                                AL.add)
                            yield
                            TG2 = wpool.tile([128, PLT * 32 * 9], f16,
                                             tag=f"TG2{it}")
                            TG2v = TG2[:].rearrange(
                                "p (ltou k) -> p ltou k", k=9)
                            nc.vector.tensor_tensor(
                                TG2v, TGv[:, :, 0:9], TGv[:, :, 9:18],
                                AL.add)
                            yield
                            s_t = smpool.tile([128, PLT * 32], f32,
                                              tag="s")
                            nc.vector.tensor_reduce(s_t[:].unsqueeze(2),
                                                    TG2v, AX.X, AL.add)
                            yield
                            if is_dbg and it == 1:
                                nc.sync.dma_start(dbg["dbg_s1"][:], s_t[:])
                            v = squash(s_t, str(it),
                                       f16 if it == 1 else f32)
                            yield
                        if it < 2:
                            # H = P1 * v_bcast (fp16 2x, u innermost);
                            # db = sum_u H as fp16 tree on DVE
                            WH = wbpool.tile([128, PLT * PF], f16, tag="W")
                            vb = v[:].rearrange(
                                "p (lt o u) -> p lt o u", lt=PLT,
                                o=O).unsqueeze(3).broadcast_to(
                                    (128, PLT, O, IJF, U))
                            nc.vector.tensor_tensor(
                                WH[:].rearrange(
                                    "p (lt o ijf u) -> p lt o ijf u",
                                    lt=PLT, o=O, ijf=IJF, u=U),
                                P4, vb, AL.mult)
                            yield
                            eng = nc.vector
                            WHv = WH[:].rearrange(
                                "p (ltoijf u) -> p ltoijf u", u=U)
                            T1 = wpool.tile([128, PLT * 144 * 4], f16,
                                            tag=f"T1_{it}")
                            T1v = T1[:].rearrange("p (k u) -> p k u", u=4)
                            eng.tensor_tensor(
                                T1v, WHv[:, :, 0:4], WHv[:, :, 4:8],
                                AL.add)
                            yield
                            T2 = wpool.tile([128, PLT * 144 * 2], f16,
                                            tag=f"T2_{it}")
                            T2v = T2[:].rearrange("p (k u) -> p k u", u=2)
                            eng.tensor_tensor(
                                T2v, T1v[:, :, 0:2], T1v[:, :, 2:4],
                                AL.add)
                            yield
                            if it == 0:
                                eng.tensor_tensor(
                                    b_t[:].unsqueeze(2),
                                    T2v[:, :, 0:1], T2v[:, :, 1:2],
                                    AL.add)
                                if is_dbg:
                                    nc.sync.dma_start(dbg["dbg_b1"][:],
                                                      b_t[:])
                            else:
                                db = smpool.tile([128, PLT * 144], f32,
                                                 tag="db")
                                nc.gpsimd.tensor_tensor(
                                    db[:].unsqueeze(2),
                                    T2v[:, :, 0:1], T2v[:, :, 1:2],
                                    AL.add)
                                nc.gpsimd.tensor_tensor(b_t[:], b_t[:],
                                                        db[:], AL.add)
                            yield

                    # v (fp32) -> transpose to [32, locs] & stage
                    for lt in range(PLT):
                        r0 = (pr * PLT + lt) * LT_ROWS
                        tp = ptp.tile([32, 128], f32, tag="tp")
                        nc.tensor.transpose(tp[:],
                                            v[:, lt * 32:(lt + 1) * 32],
                                            ident_s[:])
                        nc.scalar.copy(
                            stage[:, r0 * W:r0 * W + LT_ROWS * W], tp[:])

                # software-pipelined driver: 2 routing chains in flight,
                # interleaved per engine queue
                total_passes = NPASS * repeat
                active = []
                produced = 0
                while produced < total_passes or active:
                    if len(active) < 2 and produced < total_passes:
                        pr = produced % NPASS
                        t = produce(pr)
                        active.append(routing_gen(
                            pr, *t, dump and img == 0 and produced == 0))
                        produced += 1
                    for g in list(active):
                        try:
                            next(g)
                        except StopIteration:
                            active.remove(g)

                nc.sync.dma_start(out_d[img], stage[:])

    nc.compile()
    return nc


def _get_compiled():
    global _COMPILED
    if _COMPILED is None:
        _COMPILED = _build()
    return _COMPILED


def _make_consts(weight):
    w = np.asarray(weight, dtype=np.float32)  # [o, f, i, j, u, d]
    w_hi = w.astype(np.float16)
    w_lo = (w - w_hi.astype(np.float32)).astype(np.float16)
    wmov = np.zeros((2 * C, NIJ * 128), dtype=np.float16)
    wsum = np.zeros((2 * C, NIJ * 32), dtype=np.float16)
    for half, wh in enumerate((w_hi, w_lo)):
        whf = wh.astype(np.float32)
        ws = (0.25 * whf).astype(np.float16)
        for o in range(O):
            for f in range(F):
                for ij in range(NIJ):
                    i, j = ij // KW, ij % KW
                    for u in range(U):
                        for d in range(D):
                            # column order per tap: o*32 + f*8 + u
                            wmov[half * C + f * D + d,
                                 ij * 128 + o * 32 + f * 8 + u] = wh[
                                     o, f, i, j, u, d]
                            wsum[half * C + f * D + d,
                                 ij * 32 + o * 8 + u] = ws[o, f, i, j, u, d]
    return wmov, wsum


def kernel(x, weight):
    x = np.ascontiguousarray(np.asarray(x, dtype=np.float32))
    wmov, wsum = _make_consts(weight)
    ident = np.eye(128, dtype=np.float32)

    nc = _get_compiled()
    in_maps = []
    for c in range(N_CORES):
        xin = x[c * IMG_PER_CORE:(c + 1) * IMG_PER_CORE].reshape(
            IMG_PER_CORE, C, H * W)
        in_maps.append({
            "xin": np.ascontiguousarray(xin),
            "wmov": wmov,
            "wsum": wsum,
            "ident": ident,
        })
    res = bass_utils.run_bass_kernel_spmd(nc, in_maps,
                                          core_ids=list(range(N_CORES)))
    out = np.empty((N_FULL, C, H, W), dtype=np.float32)
    for c in range(N_CORES):
        out[c * IMG_PER_CORE:(c + 1) * IMG_PER_CORE] = res.results[c][
            "out"].reshape(IMG_PER_CORE, C, H, W)
    return out
